# revision 1
# baseline (speedup 1.0000x reference)
"""2-layer GCN (GEMM -> COO SpMM -> ReLU -> GEMM -> SpMM) on 8 trn2 NeuronCores.

Design (row-sharded, transpose-free):
  - Core m owns node rows [m*RPC, (m+1)*RPC); padded to RPAD=NB*128 rows on
    device (pad rows never referenced by gathers; dropped on host).
  - GEMM1: Z1 = X @ W1 + b1 per-core (node-major), bf16, AllGather -> Z1_full.
  - SpMM: per 128-row block, edges sorted by col, split into 4 col-quartile
    segments so gather indices fit int16 relative to a QBASE-row view of
    Z_full.  dma_gather (non-transpose) emits slot-major [128 slots, 128 f]
    chunks == matmul lhsT directly.  S[slot, row] = (iota==row_local)*val is
    one DVE tensor_scalar per chunk.  PE accumulates
    out^T[feats, rows] += G^T @ S in PSUM over a block's Q*CAP_CH chunks.
  - out^T feature-major == lhsT layout for the next GEMM (no transposes).
  - Output written feature-major [128, RPAD] f32; host transposes + trims.

SPMD: one program for 8 cores; fixed slot layout (CAP_CH chunks of 128 per
(block, quartile), padded with idx=0/val=0), per-core data varies only in
input tensors.  DMA-instruction count before fan-in points is minimized:
HW limits sync-waits per instruction and Tile round-robins each DMA onto
one of 8 HWDGE semaphore lanes.
"""

import sys

import numpy as np
import ml_dtypes

_TRN_REPO = "/opt/trn_rl_repo"
if _TRN_REPO not in sys.path:
    sys.path.insert(0, _TRN_REPO)

import concourse.bass as bass
import concourse.tile as tile
from concourse import bacc, mybir
from concourse.bass_utils import run_bass_kernel_spmd

BF16 = mybir.dt.bfloat16
F32 = mybir.dt.float32
I16 = mybir.dt.int16


class Cfg:
    def __init__(self, n_nodes, in_size, hidden, out_size,
                 cap_ch=5, group_blocks=5):
        self.M = 8
        self.NN = n_nodes
        self.IN = in_size
        self.HID = hidden
        self.OUT = out_size
        assert n_nodes % self.M == 0
        self.RPC = n_nodes // self.M          # real rows per core
        self.BL = 128
        self.NB = (self.RPC + 127) // 128
        self.RPAD = self.NB * 128             # padded rows per core
        self.NNP = self.M * self.RPAD         # padded global nodes
        self.Q = 4
        assert self.NNP % self.Q == 0
        self.QBASE = self.NNP // self.Q
        assert self.QBASE <= 32768
        self.CAP_CH = cap_ch
        self.CAP = cap_ch * 128
        self.GB = group_blocks
        self.KIN = in_size // 128
        assert in_size % 128 == 0 and hidden == 128 and out_size == 128


FULL = Cfg(100000, 256, 128, 128)


def build_plan(cfg, row, col, vals):
    row = np.asarray(row).astype(np.int64)
    col = np.asarray(col).astype(np.int64)
    vals = np.asarray(vals).astype(np.float32)
    # remap cols into padded node space
    colp = (col // cfg.RPC) * cfg.RPAD + (col % cfg.RPC)

    # adaptive per-(block, quartile) capacity: scan max segment first
    need = 0
    for m in range(cfg.M):
        sel = (row // cfg.RPC) == m
        er0 = row[sel] - m * cfg.RPC
        key = (er0 // cfg.BL) * cfg.Q + colp[sel] // cfg.QBASE
        if key.size:
            need = max(need, int(np.bincount(key.astype(np.int64)).max()))
    cap_ch = max(cfg.CAP_CH, -(-need // 128))
    if cap_ch != cfg.CAP_CH:
        cfg.CAP_CH = cap_ch
        cfg.CAP = cap_ch * 128

    groups = [list(range(g, min(g + cfg.GB, cfg.NB)))
              for g in range(0, cfg.NB, cfg.GB)]
    slot_off = {}
    insts = []  # (q, slot_offset, n_slots) per (group, quartile)
    off = 0
    for blist in groups:
        for q in range(cfg.Q):
            ioff = off
            for b in blist:
                slot_off[(b, q)] = off
                off += cfg.CAP
            insts.append((q, ioff, off - ioff))
    nslot = off
    nchunk = nslot // 128

    per_core = []
    max_seg = 0
    for m in range(cfg.M):
        sel = (row // cfg.RPC) == m
        er = (row[sel] - m * cfg.RPC).astype(np.int64)
        ec = colp[sel]
        ev = vals[sel]
        blk = er // cfg.BL
        order = np.lexsort((ec, blk))
        er, ec, ev, blk = er[order], ec[order], ev[order], blk[order]

        idx16 = np.zeros(nslot, dtype=np.int16)
        rloc = np.zeros(nslot, dtype=np.float32)
        sval = np.zeros(nslot, dtype=np.float32)

        bstart = np.searchsorted(blk, np.arange(cfg.NB + 1))
        for b in range(cfg.NB):
            i0, i1 = bstart[b], bstart[b + 1]
            ecb = ec[i0:i1]
            qsplit = np.searchsorted(ecb, np.arange(cfg.Q + 1) * cfg.QBASE)
            for q in range(cfg.Q):
                j0, j1 = i0 + qsplit[q], i0 + qsplit[q + 1]
                n = j1 - j0
                max_seg = max(max_seg, n)
                if n > cfg.CAP:
                    raise RuntimeError(
                        f"segment overflow core {m} blk {b} q {q}: "
                        f"{n} > {cfg.CAP}")
                so = slot_off[(b, q)]
                idx16[so:so + n] = (ec[j0:j1] - q * cfg.QBASE).astype(np.int16)
                rloc[so:so + n] = (er[j0:j1] - b * cfg.BL).astype(np.float32)
                sval[so:so + n] = ev[j0:j1]

        idx_w = np.tile(idx16.reshape(-1, 16).T, (8, 1))
        rloc_w = rloc.reshape(nchunk, 128).T.astype(np.float32)
        sval_w = sval.reshape(nchunk, 128).T.astype(np.float32)
        per_core.append(dict(idx=np.ascontiguousarray(idx_w),
                             rloc=np.ascontiguousarray(rloc_w),
                             sval=np.ascontiguousarray(sval_w)))
    return groups, insts, slot_off, nslot, nchunk, per_core, max_seg


def build_program(cfg, groups, insts, slot_off, nslot, nchunk):
    nc = bacc.Bacc("TRN2", target_bir_lowering=False, debug=False,
                   num_devices=cfg.M)

    xt_d = nc.dram_tensor("xt", [cfg.IN, cfg.RPAD], BF16, kind="ExternalInput")
    wcols = cfg.KIN * 128 + 128 + 4 * 128
    wpack_d = nc.dram_tensor("wpack", [128, wcols], BF16, kind="ExternalInput")
    idx_d = nc.dram_tensor("idx", [128, nslot // 16], I16, kind="ExternalInput")
    fcols = 2 * nchunk
    fpack_d = nc.dram_tensor("fpack", [128, fcols], F32, kind="ExternalInput")
    out_d = nc.dram_tensor("out", [128, cfg.RPAD], F32, kind="ExternalOutput")

    z1_loc = nc.dram_tensor("z1_loc", [cfg.RPAD, cfg.HID], BF16)
    z1_full = nc.dram_tensor("z1_full", [cfg.NNP, cfg.HID], BF16)
    z2_loc = nc.dram_tensor("z2_loc", [cfg.RPAD, cfg.OUT], BF16)
    z2_full = nc.dram_tensor("z2_full", [cfg.NNP, cfg.OUT], BF16)

    rg = [list(range(cfg.M))]

    with tile.TileContext(nc) as tc:
        from contextlib import ExitStack
        with ExitStack() as ctx:
            const = ctx.enter_context(tc.tile_pool(name="const", bufs=1))
            xt_pool = ctx.enter_context(tc.tile_pool(name="xt", bufs=8))
            gbuf_pool = ctx.enter_context(tc.tile_pool(name="gbuf", bufs=2))
            s_pool = ctx.enter_context(tc.tile_pool(name="sm", bufs=4))
            ot_pool = ctx.enter_context(tc.tile_pool(name="ot", bufs=8))
            rt_pool = ctx.enter_context(tc.tile_pool(name="rt", bufs=1))
            psum_g = ctx.enter_context(
                tc.tile_pool(name="psum_g", bufs=2, space="PSUM"))
            psum_s = ctx.enter_context(
                tc.tile_pool(name="psum_s", bufs=6, space="PSUM"))

            # ---- resident constants (3 packed loads: bounded sem fan-in) ----
            wpack_sb = const.tile([128, wcols], BF16, tag="wpack",
                                  name="wpacksb")
            nc.sync.dma_start(wpack_sb[:], wpack_d[:, :])
            w1_sb = [wpack_sb[:, k * 128:(k + 1) * 128]
                     for k in range(cfg.KIN)]
            o = cfg.KIN * 128
            w2_sb = wpack_sb[:, o:o + 128]
            b1_sb = wpack_sb[0:1, o + 128:o + 256]
            b2_sb = wpack_sb[0:1, o + 256:o + 384]
            ones_sb = wpack_sb[0:1, o + 384:o + 512]
            iota_sb = wpack_sb[:, o + 512:o + 640]
            idx_sb = const.tile([128, nslot // 16], I16, tag="idx",
                                name="idxsb")
            nc.sync.dma_start(idx_sb[:], idx_d[:, :])
            fpack_sb = const.tile([128, fcols], F32, tag="fpack",
                                  name="fpacksb")
            nc.sync.dma_start(fpack_sb[:], fpack_d[:, :])
            rloc_sb = fpack_sb[:, 0:nchunk]
            sval_sb = fpack_sb[:, nchunk:2 * nchunk]
            rt_sb = rt_pool.tile([128, cfg.RPAD], BF16, tag="rt", name="rtsb")
            zs_sb = rt_pool.tile([128, cfg.RPAD], BF16, tag="zs", name="zssb")

            def gemm(lhsT_of, w_list, bias, zdst):
                """Z[t] = lhsT_t.T @ W + 1.b ; evac into zs_sb; one DMA out."""
                for t in range(cfg.NB):
                    ps = psum_g.tile([128, 128], F32, tag="gemm_ps", name="ps")
                    for k, (lt, wk) in enumerate(zip(lhsT_of(t), w_list)):
                        nc.tensor.matmul(ps[:], lt, wk,
                                         start=(k == 0), stop=False,
                                         skip_group_check=True)
                    nc.tensor.matmul(ps[:], ones_sb, bias,
                                     start=False, stop=True,
                                     skip_group_check=True)
                    nc.scalar.copy(zs_sb[:, t * 128:(t + 1) * 128], ps[:])
                nc.gpsimd.dma_start(
                    zdst.rearrange("(t p) f -> p t f", p=128)[:, :, :],
                    zs_sb.rearrange("p (t f) -> p t f", f=128)[:, :, :])

            # ---- GEMM1 ----
            def x_lhsT(t):
                tiles = []
                for k in range(cfg.KIN):
                    xt = xt_pool.tile([128, 128], BF16, tag="xt", name="xt")
                    nc.sync.dma_start(
                        xt[:], xt_d[k * 128:(k + 1) * 128,
                                    t * 128:(t + 1) * 128])
                    tiles.append(xt[:])
                return tiles

            gemm(x_lhsT, w1_sb, b1_sb, z1_loc)
            nc.gpsimd.collective_compute(
                "AllGather", mybir.AluOpType.bypass, replica_groups=rg,
                ins=[z1_loc[:, :]], outs=[z1_full[:, :]])

            # ---- SpMM ----
            def spmm(z_full, layer):
                for gi, blist in enumerate(groups):
                    nbl = len(blist)
                    ptiles = [psum_s.tile([128, 128], F32, tag="spmm_ps",
                                          name="spmm_ps")
                              for _ in range(nbl)]

                    def pview(bi):
                        return ptiles[bi][:, :]

                    for q in range(cfg.Q):
                        qi, ioff, n = insts[gi * cfg.Q + q]
                        assert qi == q
                        gb3 = gbuf_pool.tile(
                            [128, cfg.GB * cfg.CAP // 128, 128], BF16,
                            tag="gbuf", name="gbuf")
                        gb = gb3.rearrange("p c f -> p (c f)")
                        # SWDGE ring holds ~1024 descriptors; split gathers
                        o = 0
                        while o < n:
                            nj = min(1024, n - o)
                            nc.gpsimd.dma_gather(
                                out_ap=gb3[:, o // 128:(o + nj) // 128, :],
                                in_ap=z_full[q * cfg.QBASE:
                                             (q + 1) * cfg.QBASE, :],
                                idxs_ap=idx_sb[:, (ioff + o) // 16:
                                               (ioff + o + nj) // 16],
                                num_idxs=nj, num_idxs_reg=nj,
                                elem_size=cfg.HID,
                            )
                            o += nj
                        for bi, b in enumerate(blist):
                            for c in range(cfg.CAP_CH):
                                so = slot_off[(b, q)] - ioff + c * 128
                                cg = (slot_off[(b, q)] + c * 128) // 128
                                s = s_pool.tile([128, 128], BF16, tag="s",
                                                name="s")
                                nc.vector.tensor_scalar(
                                    s[:], iota_sb,
                                    rloc_sb[:, cg:cg + 1],
                                    sval_sb[:, cg:cg + 1],
                                    mybir.AluOpType.is_equal,
                                    mybir.AluOpType.mult)
                                nc.tensor.matmul(
                                    pview(bi), gb[:, so:so + 128], s[:],
                                    start=(q == 0 and c == 0),
                                    stop=(q == cfg.Q - 1 and
                                          c == cfg.CAP_CH - 1),
                                    skip_group_check=True)
                    for pi, pt in enumerate(ptiles):
                        b0 = blist[pi]
                        nw = 128
                        r0 = b0 * 128
                        if layer == 1:
                            nc.scalar.activation(
                                rt_sb[:, r0:r0 + nw], pt[:, :nw],
                                mybir.ActivationFunctionType.Relu)
                        else:
                            ot = ot_pool.tile([128, 512], F32, tag="ot",
                                              name="ot")
                            nc.scalar.copy(ot[:, :nw], pt[:, :nw])
                            nc.sync.dma_start(out_d[:, r0:r0 + nw],
                                              ot[:, :nw])

            spmm(z1_full, 1)

            # ---- GEMM2 ----
            def rt_lhsT(t):
                return [rt_sb[:, t * 128:(t + 1) * 128]]

            gemm(rt_lhsT, [w2_sb], b2_sb, z2_loc)
            nc.gpsimd.collective_compute(
                "AllGather", mybir.AluOpType.bypass, replica_groups=rg,
                ins=[z2_loc[:, :]], outs=[z2_full[:, :]])

            spmm(z2_full, 2)

    nc.compile()
    return nc


def _prep_inputs(cfg, X, W1, b1, W2, b2, per_core, nchunk):
    bf = ml_dtypes.bfloat16
    wcols = cfg.KIN * 128 + 128 + 4 * 128
    wpack = np.zeros((128, wcols), dtype=np.float32)
    for k in range(cfg.KIN):
        wpack[:, k * 128:(k + 1) * 128] = np.asarray(W1)[k * 128:(k + 1) * 128]
    o = cfg.KIN * 128
    wpack[:, o:o + 128] = np.asarray(W2)
    wpack[0, o + 128:o + 256] = np.asarray(b1)
    wpack[0, o + 256:o + 384] = np.asarray(b2)
    wpack[0, o + 384:o + 512] = 1.0
    wpack[:, o + 512:o + 640] = np.arange(128, dtype=np.float32)[None, :]
    wpack = wpack.astype(bf)

    X = np.asarray(X).astype(np.float32)
    in_maps = []
    for m in range(cfg.M):
        xs = np.zeros((cfg.IN, cfg.RPAD), dtype=np.float32)
        xs[:, :cfg.RPC] = X[m * cfg.RPC:(m + 1) * cfg.RPC].T
        fpack = np.zeros((128, 2 * nchunk), dtype=np.float32)
        fpack[:, :nchunk] = per_core[m]["rloc"]
        fpack[:, nchunk:] = per_core[m]["sval"]
        in_maps.append(dict(
            xt=np.ascontiguousarray(xs.astype(bf)), wpack=wpack,
            idx=per_core[m]["idx"], fpack=fpack))
    return in_maps


def run(cfg, X, W1, b1, W2, b2, vals, row, col, trace=False):
    groups, insts, slot_off, nslot, nchunk, per_core, max_seg = \
        build_plan(cfg, row, col, vals)
    nc = build_program(cfg, groups, insts, slot_off, nslot, nchunk)
    in_maps = _prep_inputs(cfg, X, W1, b1, W2, b2, per_core, nchunk)
    res = run_bass_kernel_spmd(nc, in_maps, list(range(cfg.M)), trace=trace)
    outs = [np.asarray(res.results[m]["out"]).T[:cfg.RPC]
            for m in range(cfg.M)]
    out = np.concatenate(outs, axis=0).astype(np.float32)
    return out, res


def kernel(X, W1, b1, W2, b2, vals, row, col):
    out, _ = run(FULL, X, W1, b1, W2, b2, vals, row, col)
    return out



# revision 22
# speedup vs baseline: 4.0577x; 4.0577x over previous
"""2-layer GCN (GEMM -> COO SpMM -> ReLU -> GEMM -> SpMM) on 8 trn2 NeuronCores.

v2 design (one collective, X-space layer 1):
  - A@(X W1 + b1) = (A@X) W1 + deg.b1^T with deg = A@1 (host bincount).
    X is a full input replicated on every core, so layer 1 needs NO
    collective: gather X rows directly (256 bf16 feats = 512B descriptors,
    full DMA efficiency), accumulate (A@X)^T per 128-row dest block on PE
    via edge-slot scatter matmuls, then a small local GEMM + rank-1 bias.
  - relu1 [RPC rows, 128] bf16 is written row-major and AllGather'd once
    (Shared-output HBM collective = NRT fast path).
  - Layer 2 = baseline SpMM structure over relu1_full: gather by (block,
    quartile), scatter-matmul to (A relu1)^T, local GEMM2 + rank-1 bias,
    row-major f32 output (no host transpose).
  - SWDGE scratch 32KB -> 2048-descriptor gather instructions (half the
    Pool-engine desc-gen instruction overhead of the 1024 default).

PSUM budget (8 banks): scatter pool 6 (L1: 3 blocks x 2 halves, L2:
6 blocks x 1) + gemm pool 2.

SPMD: one program for 8 cores; per-core data varies only in input tensors.
Slot layout per (block, quartile) padded to CAP chunks of 128; pad slots
gather row 0 with val 0 (harmless).
"""

import sys

import numpy as np
import ml_dtypes

_TRN_REPO = "/opt/trn_rl_repo"
if _TRN_REPO not in sys.path:
    sys.path.insert(0, _TRN_REPO)

import concourse.bass as bass
import concourse.tile as tile
from concourse import bacc, mybir
from concourse.bass_utils import run_bass_kernel_spmd

BF16 = mybir.dt.bfloat16
F32 = mybir.dt.float32
I16 = mybir.dt.int16

RING = 1024          # SWDGE descriptor ring (scratch 16384 / 16)
SCRATCH = 16384


class Cfg:
    def __init__(self, n_nodes, in_size, hidden, out_size):
        self.M = 8
        self.NN = n_nodes
        self.IN = in_size
        self.HID = hidden
        self.OUT = out_size
        assert n_nodes % self.M == 0
        self.RPC = n_nodes // self.M          # real rows per core
        self.BL = 128
        self.NB = (self.RPC + 127) // 128
        self.RPAD = self.NB * 128             # padded rows per core
        self.NNP = self.M * self.RPAD         # padded global nodes
        self.Q = 4
        self.QB1 = (n_nodes + self.Q - 1) // self.Q      # X-space quartile
        self.QB2 = self.NNP // self.Q                    # padded-row quartile
        assert self.QB1 <= 32768 and self.QB2 <= 32768
        self.GB1 = 3                          # blocks/group, layer 1 (2 psum each)
        self.GB2 = 6                          # blocks/group, layer 2 (1 psum each)
        assert in_size % 128 == 0 and hidden == 128 and out_size == 128


FULL = Cfg(100000, 256, 128, 128)


def _plan_layer(cfg, er, ec, ev, qbase, gb):
    """Slot layout for one SpMM layer: edges (er=dest row local, ec=source
    index, ev=val) per core, bucketed by (dest block, source quartile).

    Fine-grained packing: each (b, q) segment gets a SHARED capacity =
    max-over-cores rounded up to 16 (idx alignment); segments concatenate
    within a (group, quartile) run (padded to x128 for chunk alignment), so
    chunks of 128 slots can span block boundaries.  Each (chunk, touched
    block) pair gets its own (rloc, sval) fpack column: slots outside the
    block get rloc=200 (never equal to iota 0..127) and sval=0.
    """
    M = len(er)
    NB = cfg.NB
    nq = cfg.Q
    # shared per-(b, q) capacities
    cnt = np.zeros((M, NB * nq), dtype=np.int64)
    for m in range(M):
        key = (er[m] // cfg.BL) * nq + np.minimum(ec[m] // qbase, nq - 1)
        cnt[m] = np.bincount(key, minlength=NB * nq)
    cap16 = ((cnt.max(axis=0) + 15) // 16 * 16).reshape(NB, nq)

    groups = [list(range(g, min(g + gb, NB))) for g in range(0, NB, gb)]
    slot_off = {}
    runs = []   # per (group, quartile): dict(q, ioff, n, insts)
    off = 0
    ninst = 0
    for blist in groups:
        for q in range(nq):
            ioff = off
            for b in blist:
                slot_off[(b, q)] = off
                off += int(cap16[b, q])
            n = -(-(off - ioff) // 128) * 128          # pad run to x128
            off = ioff + n
            # chunk -> touched blocks
            insts = []
            for c in range(n // 128):
                lo, hi = ioff + c * 128, ioff + (c + 1) * 128
                touched = [b for b in blist
                           if slot_off[(b, q)] < hi and
                           slot_off[(b, q)] + int(cap16[b, q]) > lo]
                ilist = []
                for b in touched:
                    ilist.append([blist.index(b), b, ninst, False, False])
                    ninst += 1
                insts.append(ilist)
            runs.append(dict(q=q, ioff=ioff, n=n, insts=insts))
    nslot = off
    nchunk = nslot // 128
    nfcol = ninst

    # start/stop flags: first/last instance per block in emission order
    first_seen = {}
    last_seen = {}
    for run in runs:
        for ilist in run["insts"]:
            for inst in ilist:
                b = inst[1]
                if b not in first_seen:
                    first_seen[b] = inst
                last_seen[b] = inst
    for b, inst in first_seen.items():
        inst[3] = True
    for b, inst in last_seen.items():
        inst[4] = True

    # shared owner map: slot -> owning block (-1 for run pad)
    owner = np.full(nslot, -1, dtype=np.int64)
    for (b, q), so in slot_off.items():
        owner[so:so + int(cap16[b, q])] = b

    per_core = []
    for m in range(M):
        erm, ecm, evm = er[m], ec[m], ev[m]
        blk = erm // cfg.BL
        order = np.lexsort((ecm, blk))
        erm, ecm, evm, blk = erm[order], ecm[order], evm[order], blk[order]

        idx16 = np.zeros(nslot, dtype=np.int16)
        rloc = np.full(nslot, 200.0, dtype=np.float32)
        sval = np.zeros(nslot, dtype=np.float32)

        bstart = np.searchsorted(blk, np.arange(NB + 1))
        for b in range(NB):
            i0, i1 = bstart[b], bstart[b + 1]
            ecb = ecm[i0:i1]
            qsplit = np.searchsorted(ecb, np.arange(nq + 1) * qbase)
            for q in range(nq):
                j0, j1 = i0 + qsplit[q], i0 + qsplit[q + 1]
                n = j1 - j0
                assert n <= cap16[b, q]
                so = slot_off[(b, q)]
                idx16[so:so + n] = (ecm[j0:j1] - q * qbase).astype(np.int16)
                rloc[so:so + n] = (erm[j0:j1] - b * cfg.BL).astype(np.float32)
                sval[so:so + n] = evm[j0:j1]

        # per-instance fpack columns [128, nfcol]
        rcols = np.full((128, nfcol), 200.0, dtype=np.float32)
        scols = np.zeros((128, nfcol), dtype=np.float32)
        for run in runs:
            for c, ilist in enumerate(run["insts"]):
                lo = run["ioff"] + c * 128
                wo = owner[lo:lo + 128]
                wr = rloc[lo:lo + 128]
                ws = sval[lo:lo + 128]
                for bi, b, col, _, _ in ilist:
                    sel = wo == b
                    rcols[:, col] = np.where(sel, wr, 200.0)
                    scols[:, col] = np.where(sel, ws, 0.0)

        idx_w = np.tile(idx16.reshape(-1, 16).T, (8, 1))
        per_core.append(dict(
            idx=np.ascontiguousarray(idx_w),
            rloc=np.ascontiguousarray(rcols),
            sval=np.ascontiguousarray(scols)))
    return dict(groups=groups, runs=runs, nslot=nslot, nchunk=nchunk,
                nfcol=nfcol, per_core=per_core)


def build_plan(cfg, row, col, vals):
    row = np.asarray(row).astype(np.int64)
    col = np.asarray(col).astype(np.int64)
    vals = np.asarray(vals).astype(np.float32)

    er1, ec1, ev1 = [], [], []
    er2, ec2, ev2 = [], [], []
    colp = (col // cfg.RPC) * cfg.RPAD + (col % cfg.RPC)  # padded-row space
    for m in range(cfg.M):
        sel = (row // cfg.RPC) == m
        er = (row[sel] - m * cfg.RPC).astype(np.int64)
        ev = vals[sel]
        er1.append(er); ec1.append(col[sel]); ev1.append(ev)
        er2.append(er.copy()); ec2.append(colp[sel]); ev2.append(ev.copy())

    L1 = _plan_layer(cfg, er1, ec1, ev1, cfg.QB1, cfg.GB1)
    L2 = _plan_layer(cfg, er2, ec2, ev2, cfg.QB2, cfg.GB2)

    # weighted degree deg = A@1 as a [1, RPAD] partition-0 row so that
    # deg[0:1, b*128:(b+1)*128] is a [1, 128] lhsT for the rank-1 bias matmul
    deg = np.bincount(row, weights=vals, minlength=cfg.NN).astype(np.float32)
    degt = []
    for m in range(cfg.M):
        d = np.zeros((1, cfg.RPAD), dtype=np.float32)
        d[0, :cfg.RPC] = deg[m * cfg.RPC:(m + 1) * cfg.RPC]
        degt.append(np.ascontiguousarray(d.astype(ml_dtypes.bfloat16)))
    return L1, L2, degt


def build_program(cfg, L1, L2):
    nc = bacc.Bacc("TRN2", target_bir_lowering=False, debug=False,
                   num_devices=cfg.M, dynamic_dma_scratch_size=SCRATCH)

    xg_d = nc.dram_tensor("xg", [cfg.NN, cfg.IN], BF16, kind="ExternalInput")
    # wpack columns: W1a | W1b | W2 | iota | b1 | b2 (biases on row 0)
    wcols = 6 * 128
    wpack_d = nc.dram_tensor("wpack", [128, wcols], BF16, kind="ExternalInput")
    degt_d = nc.dram_tensor("degt", [1, cfg.RPAD], BF16, kind="ExternalInput")
    nsmax = max(L1["nslot"], L2["nslot"])
    ncmax = max(L1["nfcol"], L2["nfcol"])
    idx1_d = nc.dram_tensor("idx1", [128, L1["nslot"] // 16], I16,
                            kind="ExternalInput")
    idx2_d = nc.dram_tensor("idx2", [128, L2["nslot"] // 16], I16,
                            kind="ExternalInput")
    fp1_d = nc.dram_tensor("fp1", [128, 2 * L1["nfcol"]], F32,
                           kind="ExternalInput")
    fp2_d = nc.dram_tensor("fp2", [128, 2 * L2["nfcol"]], F32,
                           kind="ExternalInput")
    out_d = nc.dram_tensor("out", [cfg.RPAD, cfg.OUT], F32,
                           kind="ExternalOutput")

    r1_loc = nc.dram_tensor("r1_loc", [cfg.RPAD, cfg.HID], BF16)
    # NOTE: addr_space="Shared" would be the fast-collective path, but the
    # axon/PJRT execution backend (fake_nrt) hangs on shared scratchpads.
    r1_full = nc.dram_tensor("r1_full", [cfg.NNP, cfg.HID], BF16)

    rg = [list(range(cfg.M))]

    with tile.TileContext(nc) as tc:
        from contextlib import ExitStack
        with ExitStack() as ctx:
            const = ctx.enter_context(tc.tile_pool(name="const", bufs=1))
            idxp = ctx.enter_context(tc.tile_pool(name="idxp", bufs=1))
            fpp = ctx.enter_context(tc.tile_pool(name="fpp", bufs=1))
            gb_pool = ctx.enter_context(tc.tile_pool(name="gb", bufs=4))
            s_pool = ctx.enter_context(tc.tile_pool(name="sm", bufs=8))
            ev_pool = ctx.enter_context(tc.tile_pool(name="ev", bufs=4))
            rt_pool = ctx.enter_context(tc.tile_pool(name="rt", bufs=1))
            ot_pool = ctx.enter_context(tc.tile_pool(name="ot", bufs=2))
            psum_s = ctx.enter_context(
                tc.tile_pool(name="psum_s", bufs=6, space="PSUM"))
            psum_g = ctx.enter_context(
                tc.tile_pool(name="psum_g", bufs=2, space="PSUM"))

            # resident constants
            wpack_sb = const.tile([128, wcols], BF16, tag="wpack", name="wp")
            nc.sync.dma_start(wpack_sb[:], wpack_d[:, :])
            w1a = wpack_sb[:, 0:128]
            w1b = wpack_sb[:, 128:256]
            w2 = wpack_sb[:, 256:384]
            iota_sb = wpack_sb[:, 384:512]
            b1r = wpack_sb[0:1, 512:640]
            b2r = wpack_sb[0:1, 640:768]
            degt_sb = const.tile([1, cfg.RPAD], BF16, tag="degt", name="dg")
            nc.sync.dma_start(degt_sb[:], degt_d[:, :])

            idx_sb = idxp.tile([128, nsmax // 16], I16, tag="idx", name="ix")
            fp_sb = fpp.tile([128, 2 * ncmax], F32, tag="fp", name="fp")

            def spmm_layer(plan, src_view_of_q, elem, halves, consume_block,
                           consume_group=None):
                """Edge-slot SpMM: per group: gathers, scatter matmuls into
                per-block psum (one per half), then consume_block(b, ptiles).
                """
                nfcol = plan["nfcol"]
                rloc_sb = fp_sb[:, 0:nfcol]
                sval_sb = fp_sb[:, nfcol:2 * nfcol]
                gbch = max(r["n"] for r in plan["runs"]) // 128
                ri = 0
                for gi, blist in enumerate(plan["groups"]):
                    ptiles = [[psum_s.tile([128, 128], F32, tag="ps",
                                           name="ps")
                               for _ in range(halves)] for _ in blist]
                    for q in range(cfg.Q):
                        run = plan["runs"][ri]
                        ri += 1
                        assert run["q"] == q
                        ioff, n = run["ioff"], run["n"]
                        gb3 = gb_pool.tile([128, gbch, elem], BF16,
                                           tag="gb", name="gb")
                        o = 0
                        while o < n:
                            nj = min(RING, n - o)
                            nc.gpsimd.dma_gather(
                                out_ap=gb3[:, o // 128:(o + nj) // 128, :],
                                in_ap=src_view_of_q(q),
                                idxs_ap=idx_sb[:, (ioff + o) // 16:
                                               (ioff + o + nj) // 16],
                                num_idxs=nj, num_idxs_reg=nj,
                                elem_size=elem,
                            )
                            o += nj
                        for c, ilist in enumerate(run["insts"]):
                            for bi, b, col, st, sp in ilist:
                                s = s_pool.tile([128, 128], BF16, tag="s",
                                                name="s")
                                nc.vector.tensor_scalar(
                                    s[:], iota_sb,
                                    rloc_sb[:, col:col + 1],
                                    sval_sb[:, col:col + 1],
                                    mybir.AluOpType.is_equal,
                                    mybir.AluOpType.mult)
                                for h in range(halves):
                                    nc.tensor.matmul(
                                        ptiles[bi][h][:, :],
                                        gb3[:, c, h * 128:(h + 1) * 128],
                                        s[:],
                                        start=st, stop=sp,
                                        skip_group_check=True)
                    for bi, b in enumerate(blist):
                        consume_block(b, ptiles[bi])
                    if consume_group is not None:
                        consume_group(blist)

            def chunked_load(dst, src, ncols, parts=4):
                step = -(-ncols // parts)
                o = 0
                while o < ncols:
                    e = min(ncols, o + step)
                    nc.sync.dma_start(dst[:, o:e], src[:, o:e])
                    o = e

            # ================= Layer 1 =================
            chunked_load(idx_sb, idx1_d, L1["nslot"] // 16)
            chunked_load(fp_sb, fp1_d, 2 * L1["nfcol"])

            r1_sb = rt_pool.tile([128, cfg.RPAD], BF16, tag="r1", name="r1")

            def x_view(q):
                lo = q * cfg.QB1
                hi = min(cfg.NN, lo + cfg.QB1)
                return xg_d[lo:hi, :]

            def consume1(b, pts):
                # (A@X)^T halves -> SBUF bf16 lhsT, GEMM1 + rank-1 bias,
                # relu -> r1_sb block
                ax0 = ev_pool.tile([128, 128], BF16, tag="ax0", name="ax0")
                ax1 = ev_pool.tile([128, 128], BF16, tag="ax1", name="ax1")
                nc.scalar.copy(ax0[:], pts[0][:, :])
                nc.scalar.copy(ax1[:], pts[1][:, :])
                ps = psum_g.tile([128, 128], F32, tag="g", name="g1")
                nc.tensor.matmul(ps[:], ax0[:], w1a, start=True, stop=False,
                                 skip_group_check=True)
                nc.tensor.matmul(ps[:], ax1[:], w1b, start=False, stop=False,
                                 skip_group_check=True)
                nc.tensor.matmul(ps[:], degt_sb[0:1, b * 128:(b + 1) * 128],
                                 b1r, start=False, stop=True,
                                 skip_group_check=True)
                nc.scalar.activation(r1_sb[:, b * 128:(b + 1) * 128], ps[:],
                                     mybir.ActivationFunctionType.Relu)

            r1_loc_r = r1_loc.rearrange("(t p) f -> p t f", p=128)
            r1_sb_r = r1_sb.rearrange("p (t f) -> p t f", f=128)

            def group1_out(blist):
                b0, b1 = blist[0], blist[-1] + 1
                nc.sync.dma_start(r1_loc_r[:, b0:b1, :], r1_sb_r[:, b0:b1, :])

            spmm_layer(L1, x_view, cfg.IN, 2, consume1, group1_out)

            nc.gpsimd.collective_compute(
                "AllGather", mybir.AluOpType.bypass, replica_groups=rg,
                ins=[r1_loc[:, :]], outs=[r1_full[:, :]])

            # ================= Layer 2 =================
            nc.sync.dma_start(idx_sb[:, :L2["nslot"] // 16], idx2_d[:, :])
            nc.sync.dma_start(fp_sb[:, :2 * L2["nfcol"]], fp2_d[:, :])

            def r1_view(q):
                return r1_full[q * cfg.QB2:(q + 1) * cfg.QB2, :]

            out_r = out_d.rearrange("(t p) f -> p t f", p=128)

            def consume2(b, pts):
                ar = ev_pool.tile([128, 128], BF16, tag="ar", name="ar")
                nc.scalar.copy(ar[:], pts[0][:, :])
                ps = psum_g.tile([128, 128], F32, tag="g", name="g2")
                nc.tensor.matmul(ps[:], ar[:], w2, start=True, stop=False,
                                 skip_group_check=True)
                nc.tensor.matmul(ps[:], degt_sb[0:1, b * 128:(b + 1) * 128],
                                 b2r, start=False, stop=True,
                                 skip_group_check=True)
                ot = ot_pool.tile([128, 128], F32, tag="ot", name="ot")
                nc.scalar.copy(ot[:], ps[:])
                nc.sync.dma_start(
                    out_r[:, b:b + 1, :],
                    ot[:].rearrange("p (t f) -> p t f", t=1))

            spmm_layer(L2, r1_view, cfg.HID, 1, consume2)

    nc.compile()
    return nc


def _prep_inputs(cfg, X, W1, b1, W2, b2, L1, L2, degt):
    bf = ml_dtypes.bfloat16
    wcols = 6 * 128
    wpack = np.zeros((128, wcols), dtype=np.float32)
    W1 = np.asarray(W1, dtype=np.float32)
    wpack[:, 0:128] = W1[0:128]
    wpack[:, 128:256] = W1[128:256]
    wpack[:, 256:384] = np.asarray(W2)
    wpack[:, 384:512] = np.arange(128, dtype=np.float32)[None, :]
    wpack[0, 512:640] = np.asarray(b1)
    wpack[0, 640:768] = np.asarray(b2)
    wpack = wpack.astype(bf)

    xg = np.ascontiguousarray(np.asarray(X, dtype=np.float32).astype(bf))
    in_maps = []
    for m in range(cfg.M):
        fp1 = np.concatenate([L1["per_core"][m]["rloc"],
                              L1["per_core"][m]["sval"]], axis=1)
        fp2 = np.concatenate([L2["per_core"][m]["rloc"],
                              L2["per_core"][m]["sval"]], axis=1)
        in_maps.append(dict(
            xg=xg, wpack=wpack, degt=degt[m],
            idx1=L1["per_core"][m]["idx"], idx2=L2["per_core"][m]["idx"],
            fp1=np.ascontiguousarray(fp1), fp2=np.ascontiguousarray(fp2)))
    return in_maps


def run(cfg, X, W1, b1, W2, b2, vals, row, col, trace=False):
    L1, L2, degt = build_plan(cfg, row, col, vals)
    nc = build_program(cfg, L1, L2)
    in_maps = _prep_inputs(cfg, X, W1, b1, W2, b2, L1, L2, degt)
    res = run_bass_kernel_spmd(nc, in_maps, list(range(cfg.M)), trace=trace)
    outs = [np.asarray(res.results[m]["out"])[:cfg.RPC] for m in range(cfg.M)]
    out = np.concatenate(outs, axis=0).astype(np.float32)
    return out, res


def kernel(X, W1, b1, W2, b2, vals, row, col):
    out, _ = run(FULL, X, W1, b1, W2, b2, vals, row, col)
    return out


# revision 38
# speedup vs baseline: 5.1024x; 1.2575x over previous
"""2-layer GCN (GEMM -> COO SpMM -> ReLU -> GEMM -> SpMM) on 8 trn2 NeuronCores.

v2 design (one collective, X-space layer 1):
  - A@(X W1 + b1) = (A@X) W1 + deg.b1^T with deg = A@1 (host bincount).
    X is a full input replicated on every core, so layer 1 needs NO
    collective: gather X rows directly (256 bf16 feats = 512B descriptors,
    full DMA efficiency), accumulate (A@X)^T per 128-row dest block on PE
    via edge-slot scatter matmuls, then a small local GEMM + rank-1 bias.
  - relu1 [RPC rows, 128] bf16 is written row-major and AllGather'd once
    (Shared-output HBM collective = NRT fast path).
  - Layer 2 = baseline SpMM structure over relu1_full: gather by (block,
    quartile), scatter-matmul to (A relu1)^T, local GEMM2 + rank-1 bias,
    row-major f32 output (no host transpose).
  - SWDGE scratch 32KB -> 2048-descriptor gather instructions (half the
    Pool-engine desc-gen instruction overhead of the 1024 default).

PSUM budget (8 banks): scatter pool 6 (L1: 3 blocks x 2 halves, L2:
6 blocks x 1) + gemm pool 2.

SPMD: one program for 8 cores; per-core data varies only in input tensors.
Slot layout per (block, quartile) padded to CAP chunks of 128; pad slots
gather row 0 with val 0 (harmless).
"""

import sys

import numpy as np
import ml_dtypes

_TRN_REPO = "/opt/trn_rl_repo"
if _TRN_REPO not in sys.path:
    sys.path.insert(0, _TRN_REPO)

import concourse.bass as bass
import concourse.tile as tile
from concourse import bacc, mybir
from concourse.bass_utils import run_bass_kernel_spmd

BF16 = mybir.dt.bfloat16
F32 = mybir.dt.float32
I16 = mybir.dt.int16

RING = 1024          # SWDGE descriptor ring (scratch 16384 / 16)
SCRATCH = 16384


class Cfg:
    def __init__(self, n_nodes, in_size, hidden, out_size):
        self.M = 8
        self.NN = n_nodes
        self.IN = in_size
        self.HID = hidden
        self.OUT = out_size
        assert n_nodes % self.M == 0
        self.RPC = n_nodes // self.M          # real rows per core
        self.BL = 128
        self.NB = (self.RPC + 127) // 128
        self.RPAD = self.NB * 128             # padded rows per core
        self.NNP = self.M * self.RPAD         # padded global nodes
        self.Q = 4
        self.QB1 = (n_nodes + self.Q - 1) // self.Q      # X-space quartile
        self.QB2 = self.NNP // self.Q                    # padded-row quartile
        assert self.QB1 <= 32768 and self.QB2 <= 32768
        self.GB1 = 3                          # blocks/group, layer 1 (2 psum each)
        self.GB2 = 6                          # blocks/group, layer 2 (1 psum each)
        assert in_size % 128 == 0 and hidden == 128 and out_size == 128


FULL = Cfg(100000, 256, 128, 128)


def _plan_layer(cfg, er, ec, ev, qbase, gb):
    """Slot layout for one SpMM layer: edges (er=dest row local, ec=source
    index, ev=val) per core, bucketed by (dest block, source quartile).

    Fine-grained packing: each (b, q) segment gets a SHARED capacity =
    max-over-cores rounded up to 16 (idx alignment); segments concatenate
    within a (group, quartile) run (padded to x128 for chunk alignment), so
    chunks of 128 slots can span block boundaries.  Each (chunk, touched
    block) pair gets its own (rloc, sval) fpack column: slots outside the
    block get rloc=200 (never equal to iota 0..127) and sval=0.
    """
    M = len(er)
    NB = cfg.NB
    nq = cfg.Q
    # shared per-(b, q) capacities
    cnt = np.zeros((M, NB * nq), dtype=np.int64)
    for m in range(M):
        key = (er[m] // cfg.BL) * nq + np.minimum(ec[m] // qbase, nq - 1)
        cnt[m] = np.bincount(key, minlength=NB * nq)
    cap16 = ((cnt.max(axis=0) + 15) // 16 * 16).reshape(NB, nq)

    groups = [list(range(g, min(g + gb, NB))) for g in range(0, NB, gb)]
    slot_off = {}
    runs = []   # per (group, quartile): dict(q, ioff, n, insts)
    off = 0
    ninst = 0
    for blist in groups:
        for q in range(nq):
            ioff = off
            for b in blist:
                slot_off[(b, q)] = off
                off += int(cap16[b, q])
            n = -(-(off - ioff) // 128) * 128          # pad run to x128
            off = ioff + n
            # chunk -> touched blocks
            insts = []
            for c in range(n // 128):
                lo, hi = ioff + c * 128, ioff + (c + 1) * 128
                touched = [b for b in blist
                           if slot_off[(b, q)] < hi and
                           slot_off[(b, q)] + int(cap16[b, q]) > lo]
                ilist = []
                for b in touched:
                    ilist.append([blist.index(b), b, ninst, False, False])
                    ninst += 1
                insts.append(ilist)
            runs.append(dict(q=q, ioff=ioff, n=n, insts=insts))
    nslot = off
    nchunk = nslot // 128
    nfcol = ninst

    # start/stop flags: first/last instance per block in emission order
    first_seen = {}
    last_seen = {}
    for run in runs:
        for ilist in run["insts"]:
            for inst in ilist:
                b = inst[1]
                if b not in first_seen:
                    first_seen[b] = inst
                last_seen[b] = inst
    for b, inst in first_seen.items():
        inst[3] = True
    for b, inst in last_seen.items():
        inst[4] = True

    # shared owner map: slot -> owning block (-1 for run pad)
    owner = np.full(nslot, -1, dtype=np.int64)
    for (b, q), so in slot_off.items():
        owner[so:so + int(cap16[b, q])] = b

    per_core = []
    for m in range(M):
        erm, ecm, evm = er[m], ec[m], ev[m]
        blk = erm // cfg.BL
        order = np.lexsort((ecm, blk))
        erm, ecm, evm, blk = erm[order], ecm[order], evm[order], blk[order]

        idx16 = np.zeros(nslot, dtype=np.int16)
        rloc = np.full(nslot, 200.0, dtype=np.float32)
        sval = np.zeros(nslot, dtype=np.float32)

        bstart = np.searchsorted(blk, np.arange(NB + 1))
        for b in range(NB):
            i0, i1 = bstart[b], bstart[b + 1]
            ecb = ecm[i0:i1]
            qsplit = np.searchsorted(ecb, np.arange(nq + 1) * qbase)
            for q in range(nq):
                j0, j1 = i0 + qsplit[q], i0 + qsplit[q + 1]
                n = j1 - j0
                assert n <= cap16[b, q]
                so = slot_off[(b, q)]
                idx16[so:so + n] = (ecm[j0:j1] - q * qbase).astype(np.int16)
                rloc[so:so + n] = (erm[j0:j1] - b * cfg.BL).astype(np.float32)
                sval[so:so + n] = evm[j0:j1]

        # per-instance fpack columns [128, nfcol]
        rcols = np.full((128, nfcol), 200.0, dtype=np.float32)
        scols = np.zeros((128, nfcol), dtype=np.float32)
        for run in runs:
            for c, ilist in enumerate(run["insts"]):
                lo = run["ioff"] + c * 128
                wo = owner[lo:lo + 128]
                wr = rloc[lo:lo + 128]
                ws = sval[lo:lo + 128]
                for bi, b, col, _, _ in ilist:
                    sel = wo == b
                    rcols[:, col] = np.where(sel, wr, 200.0)
                    scols[:, col] = np.where(sel, ws, 0.0)

        idx_w = np.tile(idx16.reshape(-1, 16).T, (8, 1))
        per_core.append(dict(
            idx=np.ascontiguousarray(idx_w),
            rloc=np.ascontiguousarray(rcols),
            sval=np.ascontiguousarray(scols)))
    return dict(groups=groups, runs=runs, nslot=nslot, nchunk=nchunk,
                nfcol=nfcol, per_core=per_core)


def build_plan(cfg, row, col, vals):
    row = np.asarray(row).astype(np.int64)
    col = np.asarray(col).astype(np.int64)
    vals = np.asarray(vals).astype(np.float32)

    er1, ec1, ev1 = [], [], []
    er2, ec2, ev2 = [], [], []
    colp = (col // cfg.RPC) * cfg.RPAD + (col % cfg.RPC)  # padded-row space
    for m in range(cfg.M):
        sel = (row // cfg.RPC) == m
        er = (row[sel] - m * cfg.RPC).astype(np.int64)
        ev = vals[sel]
        er1.append(er); ec1.append(col[sel]); ev1.append(ev)
        er2.append(er.copy()); ec2.append(colp[sel]); ev2.append(ev.copy())

    L1 = _plan_layer(cfg, er1, ec1, ev1, cfg.QB1, cfg.GB1)
    L2 = _plan_layer(cfg, er2, ec2, ev2, cfg.QB2, cfg.GB2)

    # weighted degree deg = A@1 as a [1, RPAD] partition-0 row so that
    # deg[0:1, b*128:(b+1)*128] is a [1, 128] lhsT for the rank-1 bias matmul
    deg = np.bincount(row, weights=vals, minlength=cfg.NN).astype(np.float32)
    degt = []
    for m in range(cfg.M):
        d = np.zeros((1, cfg.RPAD), dtype=np.float32)
        d[0, :cfg.RPC] = deg[m * cfg.RPC:(m + 1) * cfg.RPC]
        degt.append(np.ascontiguousarray(d.astype(ml_dtypes.bfloat16)))
    return L1, L2, degt


def build_program(cfg, L1, L2):
    nc = bacc.Bacc("TRN2", target_bir_lowering=False, debug=False,
                   num_devices=cfg.M, dynamic_dma_scratch_size=SCRATCH)

    xg_d = nc.dram_tensor("xg", [cfg.NN, cfg.IN], BF16, kind="ExternalInput")
    # wpack columns: W1a | W1b | W2 | iota | b1 | b2 (biases on row 0)
    wcols = 6 * 128
    wpack_d = nc.dram_tensor("wpack", [128, wcols], BF16, kind="ExternalInput")
    degt_d = nc.dram_tensor("degt", [1, cfg.RPAD], BF16, kind="ExternalInput")
    nsmax = max(L1["nslot"], L2["nslot"])
    ncmax = max(L1["nfcol"], L2["nfcol"])
    idx1_d = nc.dram_tensor("idx1", [128, L1["nslot"] // 16], I16,
                            kind="ExternalInput")
    idx2_d = nc.dram_tensor("idx2", [128, L2["nslot"] // 16], I16,
                            kind="ExternalInput")
    fp1_d = nc.dram_tensor("fp1", [128, 2 * L1["nfcol"]], F32,
                           kind="ExternalInput")
    fp2_d = nc.dram_tensor("fp2", [128, 2 * L2["nfcol"]], F32,
                           kind="ExternalInput")
    out_d = nc.dram_tensor("out", [cfg.RPAD, cfg.OUT], F32,
                           kind="ExternalOutput")

    r1_loc = nc.dram_tensor("r1_loc", [cfg.RPAD, cfg.HID], BF16)
    # NOTE: addr_space="Shared" would be the fast-collective path, but the
    # axon/PJRT execution backend (fake_nrt) hangs on shared scratchpads.
    r1_full = nc.dram_tensor("r1_full", [cfg.NNP, cfg.HID], BF16)

    rg = [list(range(cfg.M))]

    with tile.TileContext(nc) as tc:
        from contextlib import ExitStack
        with ExitStack() as ctx:
            const = ctx.enter_context(tc.tile_pool(name="const", bufs=1))
            idxp = ctx.enter_context(tc.tile_pool(name="idxp", bufs=1))
            fpp = ctx.enter_context(tc.tile_pool(name="fpp", bufs=1))
            gb_pool = ctx.enter_context(tc.tile_pool(name="gb", bufs=8))
            s_pool = ctx.enter_context(tc.tile_pool(name="sm", bufs=128))
            ev_pool = ctx.enter_context(tc.tile_pool(name="ev", bufs=8))
            rt_pool = ctx.enter_context(tc.tile_pool(name="rt", bufs=1))
            ot_pool = ctx.enter_context(tc.tile_pool(name="ot", bufs=2))
            psum_s = ctx.enter_context(
                tc.tile_pool(name="psum_s", bufs=6, space="PSUM"))
            psum_g = ctx.enter_context(
                tc.tile_pool(name="psum_g", bufs=2, space="PSUM"))

            # resident constants
            wpack_sb = const.tile([128, wcols], BF16, tag="wpack", name="wp")
            nc.sync.dma_start(wpack_sb[:], wpack_d[:, :])
            w1a = wpack_sb[:, 0:128]
            w1b = wpack_sb[:, 128:256]
            w2 = wpack_sb[:, 256:384]
            iota_sb = wpack_sb[:, 384:512]
            b1r = wpack_sb[0:1, 512:640]
            b2r = wpack_sb[0:1, 640:768]
            degt_sb = const.tile([1, cfg.RPAD], BF16, tag="degt", name="dg")
            nc.sync.dma_start(degt_sb[:], degt_d[:, :])

            idx_sb = idxp.tile([128, nsmax // 16], I16, tag="idx", name="ix")
            fp_sb = fpp.tile([128, 2 * ncmax], F32, tag="fp", name="fp")

            def spmm_layer(plan, src_view_of_q, elem, halves, consume_block,
                           consume_group=None):
                """Edge-slot SpMM: per group: gathers, scatter matmuls into
                per-block psum (one per half).  consume_block(b, ptiles)
                evacuates psum immediately and returns a closure with the
                trailing GEMM work; closures flush after the NEXT group's
                first run so the PE never bubbles on fresh evacuations.
                """
                nfcol = plan["nfcol"]
                rloc_sb = fp_sb[:, 0:nfcol]
                sval_sb = fp_sb[:, nfcol:2 * nfcol]
                gbch = max(r["n"] for r in plan["runs"]) // 128
                ri = 0
                pending = []
                for gi, blist in enumerate(plan["groups"]):
                    ptiles = [[psum_s.tile([128, 128], F32, tag="ps",
                                           name="ps")
                               for _ in range(halves)] for _ in blist]
                    for q in range(cfg.Q):
                        run = plan["runs"][ri]
                        ri += 1
                        assert run["q"] == q
                        ioff, n = run["ioff"], run["n"]
                        gb3 = gb_pool.tile([128, gbch, elem], BF16,
                                           tag="gb", name="gb")
                        o = 0
                        while o < n:
                            nj = min(RING, n - o)
                            nc.gpsimd.dma_gather(
                                out_ap=gb3[:, o // 128:(o + nj) // 128, :],
                                in_ap=src_view_of_q(q),
                                idxs_ap=idx_sb[:, (ioff + o) // 16:
                                               (ioff + o + nj) // 16],
                                num_idxs=nj, num_idxs_reg=nj,
                                elem_size=elem,
                            )
                            o += nj
                        for c, ilist in enumerate(run["insts"]):
                            for bi, b, col, st, sp in ilist:
                                s = s_pool.tile([128, 128], BF16,
                                                tag="s", name="s")
                                nc.vector.tensor_scalar(
                                    s[:], iota_sb,
                                    rloc_sb[:, col:col + 1],
                                    sval_sb[:, col:col + 1],
                                    mybir.AluOpType.is_equal,
                                    mybir.AluOpType.mult)
                                for h in range(halves):
                                    nc.tensor.matmul(
                                        ptiles[bi][h][:, :],
                                        gb3[:, c, h * 128:(h + 1) * 128],
                                        s[:],
                                        start=st, stop=sp,
                                        skip_group_check=True)
                        if q == 0 and pending:
                            for fn in pending:
                                fn()
                            pending = []
                    pending = [consume_block(b, ptiles[bi])
                               for bi, b in enumerate(blist)]
                    if consume_group is not None:
                        pending.append(consume_group(blist))
                for fn in pending:
                    fn()

            def chunked_load(dst, src, ncols, parts=4):
                step = -(-ncols // parts)
                o = 0
                while o < ncols:
                    e = min(ncols, o + step)
                    nc.sync.dma_start(dst[:, o:e], src[:, o:e])
                    o = e

            # ================= Layer 1 =================
            chunked_load(idx_sb, idx1_d, L1["nslot"] // 16)
            chunked_load(fp_sb, fp1_d, 2 * L1["nfcol"])

            r1_sb = rt_pool.tile([128, cfg.RPAD], BF16, tag="r1", name="r1")

            def x_view(q):
                lo = q * cfg.QB1
                hi = min(cfg.NN, lo + cfg.QB1)
                return xg_d[lo:hi, :]

            def consume1(b, pts):
                # evacuate (A@X)^T halves now (Act + DVE in parallel) so the
                # psum scatter tiles free immediately; GEMM1 + rank-1 bias +
                # relu are deferred via the returned closure
                ax0 = ev_pool.tile([128, 128], BF16, tag="ax0", name="ax0")
                ax1 = ev_pool.tile([128, 128], BF16, tag="ax1", name="ax1")
                nc.scalar.copy(ax0[:], pts[0][:, :])
                nc.vector.tensor_copy(ax1[:], pts[1][:, :])

                def gemm():
                    ps = psum_g.tile([128, 128], F32, tag="g", name="g1")
                    nc.tensor.matmul(ps[:], ax0[:], w1a, start=True,
                                     stop=False, skip_group_check=True)
                    nc.tensor.matmul(ps[:], ax1[:], w1b, start=False,
                                     stop=False, skip_group_check=True)
                    nc.tensor.matmul(ps[:],
                                     degt_sb[0:1, b * 128:(b + 1) * 128],
                                     b1r, start=False, stop=True,
                                     skip_group_check=True)
                    nc.scalar.activation(r1_sb[:, b * 128:(b + 1) * 128],
                                         ps[:],
                                         mybir.ActivationFunctionType.Relu)
                return gemm

            r1_loc_r = r1_loc.rearrange("(t p) f -> p t f", p=128)
            r1_sb_r = r1_sb.rearrange("p (t f) -> p t f", f=128)

            def group1_out(blist):
                def emit():
                    b0, b1 = blist[0], blist[-1] + 1
                    nc.sync.dma_start(r1_loc_r[:, b0:b1, :],
                                      r1_sb_r[:, b0:b1, :])
                return emit

            spmm_layer(L1, x_view, cfg.IN, 2, consume1, group1_out)

            nc.gpsimd.collective_compute(
                "AllGather", mybir.AluOpType.bypass, replica_groups=rg,
                ins=[r1_loc[:, :]], outs=[r1_full[:, :]])

            # ================= Layer 2 =================
            nc.sync.dma_start(idx_sb[:, :L2["nslot"] // 16], idx2_d[:, :])
            nc.sync.dma_start(fp_sb[:, :2 * L2["nfcol"]], fp2_d[:, :])

            def r1_view(q):
                return r1_full[q * cfg.QB2:(q + 1) * cfg.QB2, :]

            out_r = out_d.rearrange("(t p) f -> p t f", p=128)

            def consume2(b, pts):
                ar = ev_pool.tile([128, 128], BF16, tag="ar", name="ar",
                                  bufs=14)
                nc.scalar.copy(ar[:], pts[0][:, :])

                def gemm():
                    ps = psum_g.tile([128, 128], F32, tag="g", name="g2")
                    nc.tensor.matmul(ps[:], ar[:], w2, start=True, stop=False,
                                     skip_group_check=True)
                    nc.tensor.matmul(ps[:],
                                     degt_sb[0:1, b * 128:(b + 1) * 128],
                                     b2r, start=False, stop=True,
                                     skip_group_check=True)
                    ot = ot_pool.tile([128, 128], F32, tag="ot", name="ot")
                    nc.scalar.copy(ot[:], ps[:])
                    nc.sync.dma_start(
                        out_r[:, b:b + 1, :],
                        ot[:].rearrange("p (t f) -> p t f", t=1))
                return gemm

            spmm_layer(L2, r1_view, cfg.HID, 1, consume2)

    nc.compile()
    return nc


def _prep_inputs(cfg, X, W1, b1, W2, b2, L1, L2, degt):
    bf = ml_dtypes.bfloat16
    wcols = 6 * 128
    wpack = np.zeros((128, wcols), dtype=np.float32)
    W1 = np.asarray(W1, dtype=np.float32)
    wpack[:, 0:128] = W1[0:128]
    wpack[:, 128:256] = W1[128:256]
    wpack[:, 256:384] = np.asarray(W2)
    wpack[:, 384:512] = np.arange(128, dtype=np.float32)[None, :]
    wpack[0, 512:640] = np.asarray(b1)
    wpack[0, 640:768] = np.asarray(b2)
    wpack = wpack.astype(bf)

    xg = np.ascontiguousarray(np.asarray(X, dtype=np.float32).astype(bf))
    in_maps = []
    for m in range(cfg.M):
        fp1 = np.concatenate([L1["per_core"][m]["rloc"],
                              L1["per_core"][m]["sval"]], axis=1)
        fp2 = np.concatenate([L2["per_core"][m]["rloc"],
                              L2["per_core"][m]["sval"]], axis=1)
        in_maps.append(dict(
            xg=xg, wpack=wpack, degt=degt[m],
            idx1=L1["per_core"][m]["idx"], idx2=L2["per_core"][m]["idx"],
            fp1=np.ascontiguousarray(fp1), fp2=np.ascontiguousarray(fp2)))
    return in_maps


def run(cfg, X, W1, b1, W2, b2, vals, row, col, trace=False):
    L1, L2, degt = build_plan(cfg, row, col, vals)
    nc = build_program(cfg, L1, L2)
    in_maps = _prep_inputs(cfg, X, W1, b1, W2, b2, L1, L2, degt)
    res = run_bass_kernel_spmd(nc, in_maps, list(range(cfg.M)), trace=trace)
    outs = [np.asarray(res.results[m]["out"])[:cfg.RPC] for m in range(cfg.M)]
    out = np.concatenate(outs, axis=0).astype(np.float32)
    return out, res


def kernel(X, W1, b1, W2, b2, vals, row, col):
    out, _ = run(FULL, X, W1, b1, W2, b2, vals, row, col)
    return out
